# revision 4
# baseline (speedup 1.0000x reference)
"""Trainium2 Bass kernel for the CdfgReader GNN message-passing problem.

Reference computation (shapes hardcoded):
    G, N, F, H, B, L = 4, 1024, 256, 256, 32, 4
    X = batch_xs[graph_idx]          # [B, N, F]
    A = batch_as[graph_idx]          # [B, N, N]
    x = relu(X @ w_in + b_in)
    res = x
    for i in range(L-1): x = relu(A @ x @ gcn_w[i] + gcn_b[i])
    x = tanh(A @ x @ gcn_w[L-1] + gcn_b[L-1])
    x = x + res
    out[b] = masked_mean_over_nodes(x[b], cp_mask[b])   # [B, H]

Key structural insight: the whole forward up to the final masked mean depends
only on which of the G=4 distinct graphs an example selects — so we compute
the forward once per distinct graph (4 graphs) instead of once per example
(32 examples), an 8x FLOP reduction. The per-example masked mean then becomes
a tiny [B,N]x[N,H] matmul against a host-built selection matrix.

Sharding: graph-parallel — core g (g in 0..3) computes graph g's full forward
plus its [B,H] partial of the output; cores 4..7 run the same program on
zeros. The host sums the (disjoint) partials and divides by the mask counts.

Per-core device program (all matmuls bf16 with fp32 PSUM accumulation —
verified max-rel-err ~6e-4 end to end, dominated by the residual path):
    x0 = relu(XT.T @ w_in)                         16 matmuls   (lhsT = XT)
    per layer: zT = (x.T @ AT)                     32 matmuls   (lhsT = x)
               x' = act(zT.T @ W_l)                16 matmuls   (lhsT = zT)
    y = x4 + x0
    out_partial = R.T @ y                          8 matmuls    (lhsT = R)
The alternating lhsT choice (x -> zT -> x) makes the chain transpose-free.
"""

import numpy as np
import ml_dtypes

G, N, F, H, B, L = 4, 1024, 256, 256, 32, 4
N_CORES = 8
NT = N // 128          # 8 node tiles
FT = F // 128          # 2 feature tiles
HT = H // 128          # 2 hidden tiles
NCHUNK = 512           # stage-A moving free dim (one fp32 PSUM bank)

_CACHE = {}


def _split_multi_waits(nc):
    """The walrus build in this container accepts at most ONE sync wait per
    instruction, while Tile's sem-assignment emits up to ~3. Engines execute
    their instruction stream in order, so an instruction's extra waits can be
    hoisted onto same-engine NoOps inserted immediately before it."""
    import concourse.mybir as mybir

    n = 0
    for f in nc.m.functions:
        for bb in f.blocks:
            out = []
            changed = False
            for ins in bb.instructions:
                si = ins.sync_info
                if si is not None and si.on_wait and len(si.on_wait) > 1:
                    waits = list(si.on_wait)
                    for w in waits[:-1]:
                        nop = mybir.InstNoOp(
                            name=f"wsplit_{n}", engine=ins.engine)
                        n += 1
                        nop.sync_info = mybir.SyncInfo(on_wait=[w], on_update=[])
                        out.append(nop)
                    si.on_wait = [waits[-1]]
                    changed = True
                out.append(ins)
            if changed:
                bb.instructions = out
    return nc


def _build_nc(use_bias):
    import concourse.bass as bass
    import concourse.mybir as mybir

    dt = mybir.dt.bfloat16
    f32 = mybir.dt.float32
    AF = mybir.ActivationFunctionType

    nc = bass.Bass()
    # DRAM I/O (per core)
    xt_d = nc.dram_tensor("xt", [F, N], dt, kind="ExternalInput")       # X^T
    at_d = nc.dram_tensor("at", [N, N], dt, kind="ExternalInput")       # A^T
    w_in_d = nc.dram_tensor("w_in", [F, H], dt, kind="ExternalInput")
    gw_d = nc.dram_tensor("gw", [L, H, H], dt, kind="ExternalInput")
    r_d = nc.dram_tensor("r", [N, B], dt, kind="ExternalInput")         # mask^T
    if use_bias:
        # biases pre-broadcast over partitions on host: [L+1, 128, H]
        bias_d = nc.dram_tensor("bias", [L + 1, 128, H], f32, kind="ExternalInput")
    out_d = nc.dram_tensor("out", [B, H], f32, kind="ExternalOutput")

    from concourse.tile import TileContext
    with TileContext(nc) as tc:
        import contextlib

        with contextlib.ExitStack() as ctx:
            consts = ctx.enter_context(tc.tile_pool(name="consts", bufs=1))
            xpool = ctx.enter_context(tc.tile_pool(name="x", bufs=1))
            zpool = ctx.enter_context(tc.tile_pool(name="z", bufs=2))
            opool = ctx.enter_context(tc.tile_pool(name="o", bufs=2))
            psA = ctx.enter_context(tc.tile_pool(name="psA", bufs=4, space="PSUM"))
            psB = ctx.enter_context(tc.tile_pool(name="psB", bufs=4, space="PSUM"))

            # ---- loads (emit early; Tile overlaps DMA with compute) ----
            xt = [consts.tile([128, N], dt, tag=f"xt{k}", name=f"xt{k}") for k in range(FT)]
            for k in range(FT):
                nc.sync.dma_start(out=xt[k], in_=xt_d[128 * k:128 * (k + 1), :])
            w_in = [consts.tile([128, H], dt, tag=f"wi{k}", name=f"wi{k}") for k in range(FT)]
            for k in range(FT):
                nc.sync.dma_start(out=w_in[k], in_=w_in_d[128 * k:128 * (k + 1), :])
            gw = [[consts.tile([128, H], dt, tag=f"gw{i}_{k}", name=f"gw{i}_{k}") for k in range(HT)]
                  for i in range(L)]
            for i in range(L):
                for k in range(HT):
                    nc.sync.dma_start(out=gw[i][k],
                                      in_=gw_d[i, 128 * k:128 * (k + 1), :])
            at = [consts.tile([128, N], dt, tag=f"at{k}", name=f"at{k}") for k in range(NT)]
            for k in range(NT):
                nc.sync.dma_start(out=at[k], in_=at_d[128 * k:128 * (k + 1), :])
            r = [consts.tile([128, B], dt, tag=f"r{k}", name=f"r{k}") for k in range(NT)]
            for k in range(NT):
                nc.sync.dma_start(out=r[k], in_=r_d[128 * k:128 * (k + 1), :])
            if use_bias:
                bias = [consts.tile([128, H], f32, tag=f"b{i}", name=f"b{i}") for i in range(L + 1)]
                for i in range(L + 1):
                    nc.sync.dma_start(out=bias[i], in_=bias_d[i])

            # ---- input dense layer: x0 = relu(X @ w_in + b_in) ----
            x0 = [xpool.tile([128, H], dt, tag=f"x0_{m}", name=f"x0_{m}") for m in range(NT)]
            for m in range(NT):
                ps = psB.tile([128, H], f32, tag="psB", name="psB_t")
                for k in range(FT):
                    nc.tensor.matmul(ps, xt[k][:, 128 * m:128 * (m + 1)], w_in[k],
                                     start=(k == 0), stop=(k == FT - 1))
                if use_bias:
                    nc.vector.tensor_add(ps, ps, bias[0])
                nc.scalar.activation(out=x0[m], in_=ps, func=AF.Relu)

            # ---- GCN layers ----
            x_cur = x0
            for layer in range(L):
                # stage A: zT[h, dst] = sum_src x[src, h] * AT[src, dst]
                zT = [zpool.tile([128, N], dt, tag=f"zT{h}", name=f"zT_{layer}_{h}") for h in range(HT)]
                for h in range(HT):
                    for c in range(N // NCHUNK):
                        ps = psA.tile([128, NCHUNK], f32, tag="psA", name="psA_t")
                        for k in range(NT):
                            nc.tensor.matmul(
                                ps,
                                x_cur[k][:, 128 * h:128 * (h + 1)],
                                at[k][:, NCHUNK * c:NCHUNK * (c + 1)],
                                start=(k == 0), stop=(k == NT - 1))
                        nc.vector.tensor_copy(
                            out=zT[h][:, NCHUNK * c:NCHUNK * (c + 1)], in_=ps)
                # stage B: x'[dst, h'] = act(sum_h zT[h, dst] * W[h, h'] + b)
                act = AF.Tanh if layer == L - 1 else AF.Relu
                x_nxt = [xpool.tile([128, H], dt, tag=f"xn{layer % 2}_{m}", name=f"xn{layer}_{m}")
                         for m in range(NT)]
                for m in range(NT):
                    ps = psB.tile([128, H], f32, tag="psB", name="psB_t")
                    for k in range(HT):
                        nc.tensor.matmul(ps, zT[k][:, 128 * m:128 * (m + 1)],
                                         gw[layer][k],
                                         start=(k == 0), stop=(k == HT - 1))
                    if use_bias:
                        nc.vector.tensor_add(ps, ps, bias[layer + 1])
                    nc.scalar.activation(out=x_nxt[m], in_=ps, func=act)
                x_cur = x_nxt

            # ---- residual add + masked-sum matmul ----
            y = [xpool.tile([128, H], dt, tag=f"y{m}", name=f"y{m}") for m in range(NT)]
            for m in range(NT):
                nc.vector.tensor_add(y[m], x_cur[m], x0[m])
            ps = psB.tile([32, H], f32, tag="psB", name="psB_out")
            for k in range(NT):
                nc.tensor.matmul(ps, r[k][:, :], y[k],
                                 start=(k == 0), stop=(k == NT - 1))
            out_sb = opool.tile([32, H], f32, tag="out", name="out_sb")
            nc.scalar.copy(out=out_sb, in_=ps)
            nc.sync.dma_start(out=out_d[:, :], in_=out_sb)

    _split_multi_waits(nc)
    return nc


def _get_nc(use_bias):
    key = ("nc", use_bias)
    if key not in _CACHE:
        _CACHE[key] = _build_nc(use_bias)
    return _CACHE[key]


def _prepare_in_maps(batch_xs, batch_as, w_in, b_in, gcn_w, gcn_b,
                     graph_idx, cp_mask, use_bias):
    bf16 = ml_dtypes.bfloat16
    mask_f = cp_mask.astype(np.float32)                     # [B, N]
    w_in_b = np.ascontiguousarray(w_in.astype(bf16))
    gw_b = np.ascontiguousarray(gcn_w.astype(bf16))
    if use_bias:
        bias_full = np.concatenate(
            [b_in[None, :], gcn_b], axis=0).astype(np.float32)     # [L+1, H]
        bias_bcast = np.ascontiguousarray(
            np.broadcast_to(bias_full[:, None, :], (L + 1, 128, H)).copy())

    in_maps = []
    for c in range(N_CORES):
        if c < G:
            g = c
            xt = np.ascontiguousarray(batch_xs[g].T.astype(bf16))       # [F, N]
            at = np.ascontiguousarray(batch_as[g].T.astype(bf16))       # [N, N]
            sel = (graph_idx == g).astype(np.float32)[:, None] * mask_f  # [B, N]
            r = np.ascontiguousarray(sel.T.astype(bf16))                # [N, B]
        else:
            xt = np.zeros((F, N), bf16)
            at = np.zeros((N, N), bf16)
            r = np.zeros((N, B), bf16)
        m = {"xt": xt, "at": at, "w_in": w_in_b, "gw": gw_b, "r": r}
        if use_bias:
            m["bias"] = bias_bcast
        in_maps.append(m)
    return in_maps


def kernel(batch_xs, batch_as, w_in, b_in, gcn_w, gcn_b, graph_idx, cp_mask):
    from concourse import bass_utils

    batch_xs = np.asarray(batch_xs, np.float32)
    batch_as = np.asarray(batch_as, np.float32)
    w_in = np.asarray(w_in, np.float32)
    b_in = np.asarray(b_in, np.float32)
    gcn_w = np.asarray(gcn_w, np.float32)
    gcn_b = np.asarray(gcn_b, np.float32)
    graph_idx = np.asarray(graph_idx).astype(np.int64)
    cp_mask = np.asarray(cp_mask).astype(bool)

    use_bias = bool(np.any(b_in) or np.any(gcn_b))
    nc = _get_nc(use_bias)
    in_maps = _prepare_in_maps(batch_xs, batch_as, w_in, b_in, gcn_w, gcn_b,
                               graph_idx, cp_mask, use_bias)

    res = bass_utils.run_bass_kernel_spmd(nc, in_maps,
                                          core_ids=list(range(N_CORES)))

    partial = np.zeros((B, H), np.float64)
    for c in range(G):
        partial += res.results[c]["out"].astype(np.float64)
    denom = np.maximum(cp_mask.sum(axis=1, keepdims=True).astype(np.float64), 1.0)
    return (partial / denom).astype(np.float32)


# revision 6
# speedup vs baseline: 1.0346x; 1.0346x over previous
"""Trainium2 Bass kernel for the CdfgReader GNN message-passing problem.

Reference computation (shapes hardcoded):
    G, N, F, H, B, L = 4, 1024, 256, 256, 32, 4
    X = batch_xs[graph_idx]          # [B, N, F]
    A = batch_as[graph_idx]          # [B, N, N]
    x = relu(X @ w_in + b_in)
    res = x
    for i in range(L-1): x = relu(A @ x @ gcn_w[i] + gcn_b[i])
    x = tanh(A @ x @ gcn_w[L-1] + gcn_b[L-1])
    x = x + res
    out[b] = masked_mean_over_nodes(x[b], cp_mask[b])   # [B, H]

Key structural insight: the whole forward up to the final masked mean depends
only on which of the G=4 distinct graphs an example selects — so we compute
the forward once per distinct graph (4 graphs) instead of once per example
(32 examples), an 8x FLOP reduction. The per-example masked mean then becomes
a tiny [B,N]x[N,H] matmul against a host-built selection matrix.

Sharding: graph-parallel — core g (g in 0..3) computes graph g's full forward
plus its [B,H] partial of the output; cores 4..7 run the same program on
zeros. The host sums the (disjoint) partials and divides by the mask counts.

Per-core device program (all matmuls bf16 with fp32 PSUM accumulation —
verified max-rel-err ~6e-4 end to end, dominated by the residual path):
    x0 = relu(XT.T @ w_in)                         16 matmuls   (lhsT = XT)
    per layer: zT = (x.T @ AT)                     32 matmuls   (lhsT = x)
               x' = act(zT.T @ W_l)                16 matmuls   (lhsT = zT)
    y = x4 + x0
    out_partial = R.T @ y                          8 matmuls    (lhsT = R)
The alternating lhsT choice (x -> zT -> x) makes the chain transpose-free.
"""

import numpy as np
import ml_dtypes

G, N, F, H, B, L = 4, 1024, 256, 256, 32, 4
N_CORES = 8
NT = N // 128          # 8 node tiles
FT = F // 128          # 2 feature tiles
HT = H // 128          # 2 hidden tiles
NCHUNK = 512           # stage-A moving free dim (one fp32 PSUM bank)

_CACHE = {}


def _split_multi_waits(nc):
    """The walrus build in this container accepts at most ONE sync wait per
    instruction, while Tile's sem-assignment emits up to ~3. Engines execute
    their instruction stream in order, so an instruction's extra waits can be
    hoisted onto same-engine NoOps inserted immediately before it."""
    import concourse.mybir as mybir

    n = 0
    for f in nc.m.functions:
        for bb in f.blocks:
            out = []
            changed = False
            for ins in bb.instructions:
                si = ins.sync_info
                if si is not None and si.on_wait and len(si.on_wait) > 1:
                    waits = list(si.on_wait)
                    for w in waits[:-1]:
                        nop = mybir.InstNoOp(
                            name=f"wsplit_{n}", engine=ins.engine)
                        n += 1
                        nop.sync_info = mybir.SyncInfo(on_wait=[w], on_update=[])
                        out.append(nop)
                    si.on_wait = [waits[-1]]
                    changed = True
                out.append(ins)
            if changed:
                bb.instructions = out
    return nc


def _build_nc(use_bias):
    import concourse.bass as bass
    import concourse.mybir as mybir

    dt = mybir.dt.bfloat16
    f32 = mybir.dt.float32
    AF = mybir.ActivationFunctionType

    nc = bass.Bass(enable_partition_id=False)
    # DRAM I/O (per core)
    xt_d = nc.dram_tensor("xt", [F, N], dt, kind="ExternalInput")       # X^T
    at_d = nc.dram_tensor("at", [N, N], dt, kind="ExternalInput")       # A^T
    w_in_d = nc.dram_tensor("w_in", [F, H], dt, kind="ExternalInput")
    gw_d = nc.dram_tensor("gw", [L, H, H], dt, kind="ExternalInput")
    r_d = nc.dram_tensor("r", [N, B], dt, kind="ExternalInput")         # mask^T
    if use_bias:
        # biases pre-broadcast over partitions on host: [L+1, 128, H]
        bias_d = nc.dram_tensor("bias", [L + 1, 128, H], f32, kind="ExternalInput")
    out_d = nc.dram_tensor("out", [B, H], f32, kind="ExternalOutput")

    from concourse.tile import TileContext
    with TileContext(nc) as tc:
        import contextlib

        with contextlib.ExitStack() as ctx:
            consts = ctx.enter_context(tc.tile_pool(name="consts", bufs=1))
            xpool = ctx.enter_context(tc.tile_pool(name="x", bufs=1))
            zpool = ctx.enter_context(tc.tile_pool(name="z", bufs=2))
            opool = ctx.enter_context(tc.tile_pool(name="o", bufs=2))
            psA = ctx.enter_context(tc.tile_pool(name="psA", bufs=4, space="PSUM"))
            psB = ctx.enter_context(tc.tile_pool(name="psB", bufs=4, space="PSUM"))

            # ---- loads: few big DMAs (each dma_start costs ~0.6us of SP
            # issue time, so 28 small ones would serialize for ~18us) ----
            xt_b = consts.tile([128, FT, N], dt, tag="xt", name="xt_b")
            nc.sync.dma_start(out=xt_b, in_=xt_d.rearrange("(t p) n -> p t n", p=128))
            xt = [xt_b[:, k, :] for k in range(FT)]

            wi_b = consts.tile([128, FT, H], dt, tag="wi", name="wi_b")
            nc.sync.dma_start(out=wi_b, in_=w_in_d.rearrange("(t p) h -> p t h", p=128))
            w_in = [wi_b[:, k, :] for k in range(FT)]

            # A^T in two halves so layer-0 stage A can start after half 0
            at_b = consts.tile([128, NT, N], dt, tag="at", name="at_b")
            nc.sync.dma_start(
                out=at_b[:, 0:NT // 2, :],
                in_=at_d.rearrange("(t p) n -> p t n", p=128)[:, 0:NT // 2, :])
            nc.sync.dma_start(
                out=at_b[:, NT // 2:NT, :],
                in_=at_d.rearrange("(t p) n -> p t n", p=128)[:, NT // 2:NT, :])
            at = [at_b[:, k, :] for k in range(NT)]

            gw_b = consts.tile([128, L * HT, H], dt, tag="gw", name="gw_b")
            nc.sync.dma_start(
                out=gw_b, in_=gw_d.rearrange("l (t p) h -> p (l t) h", p=128))
            gw = [[gw_b[:, i * HT + k, :] for k in range(HT)] for i in range(L)]

            r_b = consts.tile([128, NT, B], dt, tag="r", name="r_b")
            nc.sync.dma_start(out=r_b, in_=r_d.rearrange("(t p) b -> p t b", p=128))
            r = [r_b[:, k, :] for k in range(NT)]
            if use_bias:
                bias = [consts.tile([128, H], f32, tag=f"b{i}", name=f"b{i}") for i in range(L + 1)]
                for i in range(L + 1):
                    nc.sync.dma_start(out=bias[i], in_=bias_d[i])

            # ---- input dense layer: x0 = relu(X @ w_in + b_in) ----
            x0 = [xpool.tile([128, H], dt, tag=f"x0_{m}", name=f"x0_{m}") for m in range(NT)]
            for m in range(NT):
                ps = psB.tile([128, H], f32, tag="psB", name="psB_t")
                for k in range(FT):
                    nc.tensor.matmul(ps, xt[k][:, 128 * m:128 * (m + 1)], w_in[k],
                                     start=(k == 0), stop=(k == FT - 1))
                if use_bias:
                    nc.vector.tensor_add(ps, ps, bias[0])
                nc.scalar.activation(out=x0[m], in_=ps, func=AF.Relu)

            # ---- GCN layers ----
            x_cur = x0
            for layer in range(L):
                # stage A: zT[h, dst] = sum_src x[src, h] * AT[src, dst]
                zT = [zpool.tile([128, N], dt, tag=f"zT{h}", name=f"zT_{layer}_{h}") for h in range(HT)]
                for h in range(HT):
                    for c in range(N // NCHUNK):
                        ps = psA.tile([128, NCHUNK], f32, tag="psA", name="psA_t")
                        for k in range(NT):
                            nc.tensor.matmul(
                                ps,
                                x_cur[k][:, 128 * h:128 * (h + 1)],
                                at[k][:, NCHUNK * c:NCHUNK * (c + 1)],
                                start=(k == 0), stop=(k == NT - 1))
                        nc.vector.tensor_copy(
                            out=zT[h][:, NCHUNK * c:NCHUNK * (c + 1)], in_=ps)
                # stage B: x'[dst, h'] = act(sum_h zT[h, dst] * W[h, h'] + b)
                act = AF.Tanh if layer == L - 1 else AF.Relu
                x_nxt = [xpool.tile([128, H], dt, tag=f"xn{layer % 2}_{m}", name=f"xn{layer}_{m}")
                         for m in range(NT)]
                for m in range(NT):
                    ps = psB.tile([128, H], f32, tag="psB", name="psB_t")
                    for k in range(HT):
                        nc.tensor.matmul(ps, zT[k][:, 128 * m:128 * (m + 1)],
                                         gw[layer][k],
                                         start=(k == 0), stop=(k == HT - 1))
                    if use_bias:
                        nc.vector.tensor_add(ps, ps, bias[layer + 1])
                    nc.scalar.activation(out=x_nxt[m], in_=ps, func=act)
                x_cur = x_nxt

            # ---- residual add + masked-sum matmul ----
            y = [xpool.tile([128, H], dt, tag=f"y{m}", name=f"y{m}") for m in range(NT)]
            for m in range(NT):
                nc.vector.tensor_add(y[m], x_cur[m], x0[m])
            ps = psB.tile([32, H], f32, tag="psB", name="psB_out")
            for k in range(NT):
                nc.tensor.matmul(ps, r[k][:, :], y[k],
                                 start=(k == 0), stop=(k == NT - 1))
            out_sb = opool.tile([32, H], f32, tag="out", name="out_sb")
            nc.scalar.copy(out=out_sb, in_=ps)
            nc.sync.dma_start(out=out_d[:, :], in_=out_sb)

    _split_multi_waits(nc)
    return nc


def _get_nc(use_bias):
    key = ("nc", use_bias)
    if key not in _CACHE:
        _CACHE[key] = _build_nc(use_bias)
    return _CACHE[key]


def _prepare_in_maps(batch_xs, batch_as, w_in, b_in, gcn_w, gcn_b,
                     graph_idx, cp_mask, use_bias):
    bf16 = ml_dtypes.bfloat16
    mask_f = cp_mask.astype(np.float32)                     # [B, N]
    w_in_b = np.ascontiguousarray(w_in.astype(bf16))
    gw_b = np.ascontiguousarray(gcn_w.astype(bf16))
    if use_bias:
        bias_full = np.concatenate(
            [b_in[None, :], gcn_b], axis=0).astype(np.float32)     # [L+1, H]
        bias_bcast = np.ascontiguousarray(
            np.broadcast_to(bias_full[:, None, :], (L + 1, 128, H)).copy())

    in_maps = []
    for c in range(N_CORES):
        if c < G:
            g = c
            xt = np.ascontiguousarray(batch_xs[g].T.astype(bf16))       # [F, N]
            at = np.ascontiguousarray(batch_as[g].T.astype(bf16))       # [N, N]
            sel = (graph_idx == g).astype(np.float32)[:, None] * mask_f  # [B, N]
            r = np.ascontiguousarray(sel.T.astype(bf16))                # [N, B]
        else:
            xt = np.zeros((F, N), bf16)
            at = np.zeros((N, N), bf16)
            r = np.zeros((N, B), bf16)
        m = {"xt": xt, "at": at, "w_in": w_in_b, "gw": gw_b, "r": r}
        if use_bias:
            m["bias"] = bias_bcast
        in_maps.append(m)
    return in_maps


def kernel(batch_xs, batch_as, w_in, b_in, gcn_w, gcn_b, graph_idx, cp_mask):
    from concourse import bass_utils

    batch_xs = np.asarray(batch_xs, np.float32)
    batch_as = np.asarray(batch_as, np.float32)
    w_in = np.asarray(w_in, np.float32)
    b_in = np.asarray(b_in, np.float32)
    gcn_w = np.asarray(gcn_w, np.float32)
    gcn_b = np.asarray(gcn_b, np.float32)
    graph_idx = np.asarray(graph_idx).astype(np.int64)
    cp_mask = np.asarray(cp_mask).astype(bool)

    use_bias = bool(np.any(b_in) or np.any(gcn_b))
    nc = _get_nc(use_bias)
    in_maps = _prepare_in_maps(batch_xs, batch_as, w_in, b_in, gcn_w, gcn_b,
                               graph_idx, cp_mask, use_bias)

    res = bass_utils.run_bass_kernel_spmd(nc, in_maps,
                                          core_ids=list(range(N_CORES)))

    partial = np.zeros((B, H), np.float64)
    for c in range(G):
        partial += res.results[c]["out"].astype(np.float64)
    denom = np.maximum(cp_mask.sum(axis=1, keepdims=True).astype(np.float64), 1.0)
    return (partial / denom).astype(np.float32)


# revision 8
# speedup vs baseline: 1.0473x; 1.0123x over previous
"""Trainium2 Bass kernel for the CdfgReader GNN message-passing problem.

Reference computation (shapes hardcoded):
    G, N, F, H, B, L = 4, 1024, 256, 256, 32, 4
    X = batch_xs[graph_idx]          # [B, N, F]
    A = batch_as[graph_idx]          # [B, N, N]
    x = relu(X @ w_in + b_in)
    res = x
    for i in range(L-1): x = relu(A @ x @ gcn_w[i] + gcn_b[i])
    x = tanh(A @ x @ gcn_w[L-1] + gcn_b[L-1])
    x = x + res
    out[b] = masked_mean_over_nodes(x[b], cp_mask[b])   # [B, H]

Key structural insight: the whole forward up to the final masked mean depends
only on which of the G=4 distinct graphs an example selects — so we compute
the forward once per distinct graph (4 graphs) instead of once per example
(32 examples), an 8x FLOP reduction. The per-example masked mean then becomes
a tiny [B,N]x[N,H] matmul against a host-built selection matrix.

Sharding: graph-parallel — core g (g in 0..3) computes graph g's full forward
plus its [B,H] partial of the output; cores 4..7 run the same program on
zeros. The host sums the (disjoint) partials and divides by the mask counts.

Per-core device program (all matmuls bf16 with fp32 PSUM accumulation —
verified max-rel-err ~6e-4 end to end, dominated by the residual path):
    x0 = relu(XT.T @ w_in)                         16 matmuls   (lhsT = XT)
    per layer: zT = (x.T @ AT)                     32 matmuls   (lhsT = x)
               x' = act(zT.T @ W_l)                16 matmuls   (lhsT = zT)
    y = x4 + x0
    out_partial = R.T @ y                          8 matmuls    (lhsT = R)
The alternating lhsT choice (x -> zT -> x) makes the chain transpose-free.
"""

import numpy as np
import ml_dtypes

G, N, F, H, B, L = 4, 1024, 256, 256, 32, 4
N_CORES = 8
NT = N // 128          # 8 node tiles
FT = F // 128          # 2 feature tiles
HT = H // 128          # 2 hidden tiles
NCHUNK = 512           # stage-A moving free dim (one fp32 PSUM bank)

_CACHE = {}


def _split_multi_waits(nc):
    """The walrus build in this container accepts at most ONE sync wait per
    instruction, while Tile's sem-assignment emits up to ~3. Engines execute
    their instruction stream in order, so an instruction's extra waits can be
    hoisted onto same-engine NoOps inserted immediately before it."""
    import concourse.mybir as mybir

    n = 0
    for f in nc.m.functions:
        for bb in f.blocks:
            out = []
            changed = False
            for ins in bb.instructions:
                si = ins.sync_info
                if si is not None and si.on_wait and len(si.on_wait) > 1:
                    waits = list(si.on_wait)
                    for w in waits[:-1]:
                        nop = mybir.InstNoOp(
                            name=f"wsplit_{n}", engine=ins.engine)
                        n += 1
                        nop.sync_info = mybir.SyncInfo(on_wait=[w], on_update=[])
                        out.append(nop)
                    si.on_wait = [waits[-1]]
                    changed = True
                out.append(ins)
            if changed:
                bb.instructions = out
    return nc


def _build_nc(use_bias):
    import concourse.bass as bass
    import concourse.mybir as mybir

    dt = mybir.dt.bfloat16
    f32 = mybir.dt.float32
    AF = mybir.ActivationFunctionType

    nc = bass.Bass(enable_partition_id=False)
    # DRAM I/O (per core)
    xt_d = nc.dram_tensor("xt", [F, N], dt, kind="ExternalInput")       # X^T
    at_d = nc.dram_tensor("at", [N, N], dt, kind="ExternalInput")       # A^T
    w_in_d = nc.dram_tensor("w_in", [F, H], dt, kind="ExternalInput")
    gw_d = nc.dram_tensor("gw", [L, H, H], dt, kind="ExternalInput")
    r_d = nc.dram_tensor("r", [N, B], dt, kind="ExternalInput")         # mask^T
    if use_bias:
        # biases pre-broadcast over partitions on host: [L+1, 128, H]
        bias_d = nc.dram_tensor("bias", [L + 1, 128, H], f32, kind="ExternalInput")
    out_d = nc.dram_tensor("out", [B, H], f32, kind="ExternalOutput")

    from concourse.tile import TileContext
    with TileContext(nc) as tc:
        import contextlib

        with contextlib.ExitStack() as ctx:
            consts = ctx.enter_context(tc.tile_pool(name="consts", bufs=1))
            xpool = ctx.enter_context(tc.tile_pool(name="x", bufs=1))
            zpool = ctx.enter_context(tc.tile_pool(name="z", bufs=2))
            opool = ctx.enter_context(tc.tile_pool(name="o", bufs=2))
            psA = ctx.enter_context(tc.tile_pool(name="psA", bufs=4, space="PSUM"))
            psB = ctx.enter_context(tc.tile_pool(name="psB", bufs=4, space="PSUM"))

            # ---- loads: few big DMAs (each dma_start costs ~0.6us of SP
            # issue time, so 28 small ones would serialize for ~18us) ----
            xt_b = consts.tile([128, FT, N], dt, tag="xt", name="xt_b")
            nc.sync.dma_start(out=xt_b, in_=xt_d.rearrange("(t p) n -> p t n", p=128))
            xt = [xt_b[:, k, :] for k in range(FT)]

            wi_b = consts.tile([128, FT, H], dt, tag="wi", name="wi_b")
            nc.sync.dma_start(out=wi_b, in_=w_in_d.rearrange("(t p) h -> p t h", p=128))
            w_in = [wi_b[:, k, :] for k in range(FT)]

            # A^T in 4 independent tiles (separate dep-tracking units) so
            # layer-0 stage A starts as soon as the first group lands
            at_g = []
            for g in range(4):
                t = consts.tile([128, 2, N], dt, tag=f"at{g}", name=f"at_g{g}")
                nc.sync.dma_start(
                    out=t,
                    in_=at_d.rearrange("(t p) n -> p t n", p=128)[:, 2 * g:2 * g + 2, :])
                at_g.append(t)
            at = [at_g[k // 2][:, k % 2, :] for k in range(NT)]

            gw_b = consts.tile([128, L * HT, H], dt, tag="gw", name="gw_b")
            nc.sync.dma_start(
                out=gw_b, in_=gw_d.rearrange("l (t p) h -> p (l t) h", p=128))
            gw = [[gw_b[:, i * HT + k, :] for k in range(HT)] for i in range(L)]

            r_b = consts.tile([128, NT, B], dt, tag="r", name="r_b")
            nc.sync.dma_start(out=r_b, in_=r_d.rearrange("(t p) b -> p t b", p=128))
            r = [r_b[:, k, :] for k in range(NT)]
            if use_bias:
                bias = [consts.tile([128, H], f32, tag=f"b{i}", name=f"b{i}") for i in range(L + 1)]
                for i in range(L + 1):
                    nc.sync.dma_start(out=bias[i], in_=bias_d[i])

            # ---- input dense layer: x0 = relu(X @ w_in + b_in) ----
            x0 = [xpool.tile([128, H], dt, tag=f"x0_{m}", name=f"x0_{m}") for m in range(NT)]
            for m in range(NT):
                ps = psB.tile([128, H], f32, tag="psB", name="psB_t")
                for k in range(FT):
                    nc.tensor.matmul(ps, xt[k][:, 128 * m:128 * (m + 1)], w_in[k],
                                     start=(k == 0), stop=(k == FT - 1))
                if use_bias:
                    nc.vector.tensor_add(ps, ps, bias[0])
                nc.scalar.activation(out=x0[m], in_=ps, func=AF.Relu)

            # ---- GCN layers ----
            NC = N // NCHUNK      # dst chunks per row (2)
            MPC = NCHUNK // 128   # node tiles per chunk (4)
            x_cur = x0
            for layer in range(L):
                # stage A: zT[h, dst] = sum_src x[src, h] * AT[src, dst].
                # c (dst chunk) is the outer loop and each (h, c) gets its own
                # SBUF tile, so stage B's m-tiles in chunk c unblock while
                # stage A still streams chunk c+1 on the PE — no PE bubble
                # waiting on the PSUM->SBUF copies.
                zT = [[zpool.tile([128, NCHUNK], dt, tag=f"zT{h}_{c}",
                                  name=f"zT_{layer}_{h}_{c}")
                       for c in range(NC)] for h in range(HT)]
                for c in range(NC):
                    for h in range(HT):
                        ps = psA.tile([128, NCHUNK], f32, tag="psA", name="psA_t")
                        for k in range(NT):
                            nc.tensor.matmul(
                                ps,
                                x_cur[k][:, 128 * h:128 * (h + 1)],
                                at[k][:, NCHUNK * c:NCHUNK * (c + 1)],
                                start=(k == 0), stop=(k == NT - 1))
                        nc.vector.tensor_copy(out=zT[h][c], in_=ps)
                # stage B: x'[dst, h'] = act(sum_h zT[h, dst] * W[h, h'] + b)
                act = AF.Tanh if layer == L - 1 else AF.Relu
                x_nxt = [xpool.tile([128, H], dt, tag=f"xn{layer % 2}_{m}", name=f"xn{layer}_{m}")
                         for m in range(NT)]
                for m in range(NT):
                    c, mc = m // MPC, m % MPC
                    ps = psB.tile([128, H], f32, tag="psB", name="psB_t")
                    for k in range(HT):
                        nc.tensor.matmul(ps, zT[k][c][:, 128 * mc:128 * (mc + 1)],
                                         gw[layer][k],
                                         start=(k == 0), stop=(k == HT - 1))
                    if use_bias:
                        nc.vector.tensor_add(ps, ps, bias[layer + 1])
                    nc.scalar.activation(out=x_nxt[m], in_=ps, func=act)
                x_cur = x_nxt

            # ---- residual add + masked-sum matmul ----
            y = [xpool.tile([128, H], dt, tag=f"y{m}", name=f"y{m}") for m in range(NT)]
            for m in range(NT):
                nc.vector.tensor_add(y[m], x_cur[m], x0[m])
            ps = psB.tile([32, H], f32, tag="psB", name="psB_out")
            for k in range(NT):
                nc.tensor.matmul(ps, r[k][:, :], y[k],
                                 start=(k == 0), stop=(k == NT - 1))
            out_sb = opool.tile([32, H], f32, tag="out", name="out_sb")
            nc.scalar.copy(out=out_sb, in_=ps)
            nc.sync.dma_start(out=out_d[:, :], in_=out_sb)

    _split_multi_waits(nc)
    return nc


def _get_nc(use_bias):
    key = ("nc", use_bias)
    if key not in _CACHE:
        _CACHE[key] = _build_nc(use_bias)
    return _CACHE[key]


def _prepare_in_maps(batch_xs, batch_as, w_in, b_in, gcn_w, gcn_b,
                     graph_idx, cp_mask, use_bias):
    bf16 = ml_dtypes.bfloat16
    mask_f = cp_mask.astype(np.float32)                     # [B, N]
    w_in_b = np.ascontiguousarray(w_in.astype(bf16))
    gw_b = np.ascontiguousarray(gcn_w.astype(bf16))
    if use_bias:
        bias_full = np.concatenate(
            [b_in[None, :], gcn_b], axis=0).astype(np.float32)     # [L+1, H]
        bias_bcast = np.ascontiguousarray(
            np.broadcast_to(bias_full[:, None, :], (L + 1, 128, H)).copy())

    in_maps = []
    for c in range(N_CORES):
        if c < G:
            g = c
            xt = np.ascontiguousarray(batch_xs[g].T.astype(bf16))       # [F, N]
            at = np.ascontiguousarray(batch_as[g].T.astype(bf16))       # [N, N]
            sel = (graph_idx == g).astype(np.float32)[:, None] * mask_f  # [B, N]
            r = np.ascontiguousarray(sel.T.astype(bf16))                # [N, B]
        else:
            xt = np.zeros((F, N), bf16)
            at = np.zeros((N, N), bf16)
            r = np.zeros((N, B), bf16)
        m = {"xt": xt, "at": at, "w_in": w_in_b, "gw": gw_b, "r": r}
        if use_bias:
            m["bias"] = bias_bcast
        in_maps.append(m)
    return in_maps


def kernel(batch_xs, batch_as, w_in, b_in, gcn_w, gcn_b, graph_idx, cp_mask):
    from concourse import bass_utils

    batch_xs = np.asarray(batch_xs, np.float32)
    batch_as = np.asarray(batch_as, np.float32)
    w_in = np.asarray(w_in, np.float32)
    b_in = np.asarray(b_in, np.float32)
    gcn_w = np.asarray(gcn_w, np.float32)
    gcn_b = np.asarray(gcn_b, np.float32)
    graph_idx = np.asarray(graph_idx).astype(np.int64)
    cp_mask = np.asarray(cp_mask).astype(bool)

    use_bias = bool(np.any(b_in) or np.any(gcn_b))
    nc = _get_nc(use_bias)
    in_maps = _prepare_in_maps(batch_xs, batch_as, w_in, b_in, gcn_w, gcn_b,
                               graph_idx, cp_mask, use_bias)

    res = bass_utils.run_bass_kernel_spmd(nc, in_maps,
                                          core_ids=list(range(N_CORES)))

    partial = np.zeros((B, H), np.float64)
    for c in range(G):
        partial += res.results[c]["out"].astype(np.float64)
    denom = np.maximum(cp_mask.sum(axis=1, keepdims=True).astype(np.float64), 1.0)
    return (partial / denom).astype(np.float32)
